# revision 35
# baseline (speedup 1.0000x reference)
"""Multi-head causal attention (B=4, S=2048, E=1024, H=16, Dh=64) on 8 TRN2
NeuronCores.

Sharding: core c -> batch b = c//2, head group hb = c%2 (8 heads each).
Each core computes Q/K/V projections for its 8 heads, causal softmax
attention, and a partial output projection over its 512 of the 1024
concat-head dims; a pairwise chunked ReduceScatter(add) sums the two head
groups, so core c returns 8 chunks of 128 sequence rows.

Schedule (single fused pipeline, PE kept continuously busy so the DVFS
p-state stays at max):
  P0, P1 (projection s-quarters) -> att(1) [P2 thunks interleaved] ->
  att(2) [P3 thunks + outproj(1) pieces] -> att(3) [outproj(2) pieces] ->
  att(0) [outproj(3) pieces] -> outproj(0) + final chunked RS (small tail).

Layouts on device:
  xT   [E=1024, S=2048] f32r (host-pretransposed x[b])
  QT   [128, S] bf16 per head: rows po..po+63 = Q_h^T, other 64 rows zero
       so every scores matmul contracts over K=128
  KT   [128 (2 heads x 64d), S] bf16 x 4 tiles
  V    [s-tile 128, 8 heads x 65] bf16 (65th col = ones -> softmax denom)
  scoresT [k-tile 128, q 512] f32 psum = KT_pair.T @ QT_pad; exp on ACT
  attn [128, 1024] bf16 (exp output; diag band masked by bf16 tri mult,
       4x DVE mode)
  ctxT [65, q 512] f32 psum accum over k-tiles = V_aug.T @ attnT
       (row 64 = denom)
  out  [s-tile 128, e 512] accum over 4 f-tiles = ctxT.T @ Wo (f32r)

Softmax runs unnormalized; normalization is batched per head-pair: two
reciprocals read the denom rows straight from PSUM, one PE matmul
broadcasts both recips to 128 partitions via a [2,128] selector, then two
[64,512] DVE mults write the normalized ctx tiles (f32r).
"""

import numpy as np

B, S, E = 4, 2048, 1024
H, Dh = 16, 64
HL = 8          # heads per core
N_CORES = 8
SC = 0.125      # 1/sqrt(Dh)

_CACHE = {}


def _build():
    import concourse.bacc as bacc
    import concourse.mybir as mybir
    import concourse.tile as tile

    F32 = mybir.dt.float32
    F32R = mybir.dt.float32r
    BF16 = mybir.dt.bfloat16
    Exp = mybir.ActivationFunctionType.Exp
    mult = mybir.AluOpType.mult
    add = mybir.AluOpType.add

    nc = bacc.Bacc("TRN2", target_bir_lowering=False, debug=False)

    xT = nc.dram_tensor("xT", [E, S], F32R, kind="ExternalInput")
    wq = nc.dram_tensor("wq", [E, 512], F32R, kind="ExternalInput")
    wk = nc.dram_tensor("wk", [E, 512], F32R, kind="ExternalInput")
    wv = nc.dram_tensor("wv", [E, 512], F32R, kind="ExternalInput")
    wo = nc.dram_tensor("wo", [512, E], F32R, kind="ExternalInput")
    bqk = nc.dram_tensor("bqk", [128, 8], F32, kind="ExternalInput")
    bvb = nc.dram_tensor("bvb", [128, 512], F32, kind="ExternalInput")
    bob = nc.dram_tensor("bob", [128, E], F32, kind="ExternalInput")
    maskt = nc.dram_tensor("maskt", [128, 128], BF16, kind="ExternalInput")
    e0sel = nc.dram_tensor("e0sel", [128, 64], F32R, kind="ExternalInput")
    out = nc.dram_tensor("out", [S // 2, E], F32, kind="ExternalOutput")

    NST = S // 128          # 16 s-tiles of 128
    NET = E // 128          # 8 e-tiles
    QBS = [1, 2, 3, 0]      # q-block processing order (small block last)

    with tile.TileContext(nc) as tc:
        with (
            tc.tile_pool(name="persist", bufs=1) as pp,
            tc.tile_pool(name="psum", bufs=1, space="PSUM") as psp,
            tc.tile_pool(name="dram", bufs=1, space="DRAM") as dram,
            tc.tile_pool(name="ph1", bufs=1) as p1,
            tc.tile_pool(name="ph2", bufs=1) as p2,
        ):
            # ---- persistent tiles ----
            qt8 = [pp.tile([128, S], BF16, tag="qt", bufs=8, name=f"qt{i}")
                   for i in range(HL)]
            kt_t = [pp.tile([128, S], BF16, tag="kt", bufs=4, name=f"kt{i}")
                    for i in range(4)]
            v_t = [pp.tile([128, HL * 65], BF16, tag="v", bufs=NST,
                           name=f"v{i}") for i in range(NST)]

            bqk_sb = pp.tile([128, 8], F32, tag="bqk")
            bvb_sb = pp.tile([128, 512], F32, tag="bvb")
            tri_sb = pp.tile([128, 128], BF16, tag="mask", name="tri")
            e0_sb = pp.tile([128, 64], F32R, tag="e0", name="e0")
            rfull = [pp.tile([128, 512], F32R, tag=f"rfull{i}",
                             name=f"rfull{i}") for i in range(2)]
            bob_sb = pp.tile([128, E], F32, tag="bob", name="bob")
            wos = [pp.tile([128, E], F32R, tag="wo", bufs=4, name=f"wo{ft}")
                   for ft in range(4)]

            partials = [dram.tile([256, E], BF16, tag="partial", bufs=8,
                                  name=f"partial{i}") for i in range(8)]
            rs_outs = [dram.tile([128, E], BF16, tag="rsout", bufs=8,
                                 name=f"rsout{i}") for i in range(8)]
            RGROUPS = [[0, 1], [2, 3], [4, 5], [6, 7]]

            # ---- DMA helpers ----
            def x_dma(qt, eng=None):
                """Issue the 8 xT tile DMAs for s-quarter qt."""
                if eng is None:
                    eng = nc.sync
                s0 = qt * 512
                tiles = []
                for e in range(NET):
                    t = p1.tile([128, 512], F32R, tag="xt", bufs=12,
                                name=f"xt{qt}_{e}")
                    eng.dma_start(
                        t[:], xT.ap()[e * 128:(e + 1) * 128, s0:s0 + 512])
                    tiles.append(t)
                return tiles

            # first quarter's x, interleaved with wq (consumed first)
            wt = {0: [], 1: [], 2: []}
            xts_q = {}
            s0 = 0
            xts_q[0] = []
            for e in range(NET):
                t = p1.tile([128, 512], F32R, tag="xt", bufs=12,
                            name=f"xt0_{e}")
                w = p1.tile([128, 512], F32R, tag="w", bufs=24,
                            name=f"w0_{e}")
                nc.sync.dma_start(t[:], xT.ap()[e * 128:(e + 1) * 128, 0:512])
                nc.sync.dma_start(w[:], wq.ap()[e * 128:(e + 1) * 128, :])
                xts_q[0].append(t)
                wt[0].append(w)
            nc.sync.dma_start(bqk_sb[:], bqk.ap())
            nc.sync.dma_start(bvb_sb[:], bvb.ap())
            # wk/wv ride the scalar/vector engine DMA queues so the sync
            # queues stay dedicated to the x stream (startup is DMA-bound)
            for wi, wd in [(1, wk), (2, wv)]:
                for e in range(NET):
                    w = p1.tile([128, 512], F32R, tag="w", bufs=24,
                                name=f"w{wi}_{e}")
                    eng = nc.scalar if e < 4 else nc.gpsimd
                    eng.dma_start(w[:], wd.ap()[e * 128:(e + 1) * 128, :])
                    wt[wi].append(w)
            xts_q[1] = x_dma(1)
            # ones columns for every V tile (written once on DVE; V thunks
            # fill the other columns)
            for st in range(NST):
                v3 = v_t[st][:].rearrange("p (h d) -> p h d", h=HL, d=65)
                nc.vector.memset(v3[:, :, 64:65], 1.0)
            nc.scalar.dma_start(tri_sb[:], maskt.ap())
            nc.scalar.dma_start(e0_sb[:], e0sel.ap())
            nc.scalar.dma_start(bob_sb[:], bob.ap())
            for ft in range(4):
                nc.scalar.dma_start(wos[ft][:],
                                    wo.ap()[ft * 128:(ft + 1) * 128, :])

            # zero the pad rows of the per-head QT tiles (DVE is idle at
            # kernel start; gpsimd must stay free for its wk/wv DMAs)
            for h in range(HL):
                pad = 64 - 64 * (h & 1)   # the *other* 64 rows
                nc.vector.memset(qt8[h][pad:pad + 64, :], 0.0)

            # ---- projection emitters ----
            def proj_q_tile(qt, t4, xts):
                cols = slice(qt * 512, qt * 512 + 512)
                ws = wt[0]
                ps = psp.tile([128, 1024], F32, tag="sc", bufs=3,
                              name="accp")
                for e in range(NET):
                    nc.tensor.matmul(
                        ps[:, 0:512],
                        ws[e][:, t4 * 128:(t4 + 1) * 128],
                        xts[e][:],
                        start=(e == 0), stop=(e == NET - 1))
                with nc.allow_low_precision(reason="bf16"):
                    nc.vector.tensor_scalar_add(
                        qt8[2 * t4][0:64, cols], ps[0:64, 0:512],
                        bqk_sb[0:64, t4:t4 + 1])
                    nc.vector.tensor_scalar_add(
                        qt8[2 * t4 + 1][64:128, cols], ps[64:128, 0:512],
                        bqk_sb[64:128, t4:t4 + 1])

            def proj_k_tile(qt, t4, xts):
                cols = slice(qt * 512, qt * 512 + 512)
                ws = wt[1]
                ps = psp.tile([128, 1024], F32, tag="sc", bufs=3,
                              name="accp")
                for e in range(NET):
                    nc.tensor.matmul(
                        ps[:, 0:512],
                        ws[e][:, t4 * 128:(t4 + 1) * 128],
                        xts[e][:],
                        start=(e == 0), stop=(e == NET - 1))
                with nc.allow_low_precision(reason="bf16"):
                    nc.vector.tensor_scalar_add(
                        kt_t[t4][:, cols], ps[:, 0:512],
                        bqk_sb[:, 4 + t4:5 + t4])

            def proj_v_tile(qt, st, xts):
                gst = qt * 4 + st
                ws = wt[2]
                ps = psp.tile([128, 1024], F32, tag="sc", bufs=3,
                              name="accp")
                for e in range(NET):
                    nc.tensor.matmul(
                        ps[:, 0:512],
                        xts[e][:, st * 128:(st + 1) * 128],
                        ws[e][:],
                        start=(e == 0), stop=(e == NET - 1))
                vt = v_t[gst]
                v3 = vt[:].rearrange("p (h d) -> p h d", h=HL, d=65)
                with nc.allow_low_precision(reason="bf16"):
                    nc.vector.tensor_tensor(
                        out=v3[:, :, 0:64],
                        in0=ps[:, 0:512].rearrange(
                            "p (h d) -> p h d", h=HL, d=64),
                        in1=bvb_sb[:].rearrange(
                            "p (h d) -> p h d", h=HL, d=64),
                        op=add)

            def proj_thunks(qt, xts):
                th = []
                for t4 in range(4):
                    th.append(lambda qt=qt, t4=t4: proj_q_tile(qt, t4, xts))
                for t4 in range(4):
                    th.append(lambda qt=qt, t4=t4: proj_k_tile(qt, t4, xts))
                for st in range(4):
                    th.append(lambda qt=qt, st=st: proj_v_tile(qt, st, xts))
                return th

            # ---- eager quarters 0 and 1 ----
            for thunk in proj_thunks(0, xts_q[0]):
                thunk()
            # zero rfull rows once (garbage x 0 via a clean source; memset
            # can't write float32r)
            for t in rfull:
                with nc.allow_low_precision(reason="f32r"):
                    nc.vector.tensor_scalar_mul(t[:], xts_q[0][0][:], 0.0)
            for thunk in proj_thunks(1, xts_q[1]):
                thunk()

            # ---- attention + outproj pipeline ----
            pending = [None]              # deferred normalization
            nblk = [0]

            def normalize():
                po, ctxd, ctx_ps = pending[0]
                rf = rfull[nblk[0] % 2]
                nblk[0] += 1
                dn = p2.tile([1, 512], F32, tag="dn", bufs=1, name="dn")
                nc.vector.tensor_copy(dn[:], ctx_ps[64:65, :])
                rc = p2.tile([1, 512], F32, tag="rc", bufs=1, name="rc")
                nc.vector.reciprocal_approx_fast(out=rc[:], in_=dn[:])
                with nc.allow_low_precision(reason="f32r"):
                    nc.vector.tensor_copy(rf[0:1, :], rc[:])
                rb_ps = psp.tile([128, 1024], F32, tag="sc", bufs=3,
                                 name="rbp")
                nc.tensor.matmul(rb_ps[0:64, 0:512], e0_sb[:], rf[:],
                                 start=True, stop=True)
                recip_b = p2.tile([64, 512], F32, tag="recipb", bufs=1,
                                  name="recipb")
                nc.vector.tensor_copy(recip_b[:], rb_ps[0:64, 0:512])
                with nc.allow_low_precision(reason="f32r"):
                    nc.vector.tensor_tensor(
                        out=ctxd[po:po + 64, :],
                        in0=ctx_ps[0:64, :], in1=recip_b[:], op=mult)
                pending[0] = None

            def outproj_piece(qb, ctx4, stl):
                ck = 2 * qb + stl // 2
                for eh in range(2):
                    ps = psp.tile([128, 1024], F32, tag="sc", bufs=3,
                                  name="accp3")
                    for ft in range(4):
                        nc.tensor.matmul(
                            ps[:, 0:512],
                            ctx4[ft][:, stl * 128:(stl + 1) * 128],
                            wos[ft][:, eh * 512:(eh + 1) * 512],
                            start=(ft == 0), stop=(ft == 3))
                    ob = p2.tile([128, 512], BF16, tag="outp", bufs=3,
                                 name="outp")
                    with nc.allow_low_precision(reason="bf16"):
                        nc.vector.tensor_tensor(
                            out=ob[:], in0=ps[:, 0:512],
                            in1=bob_sb[:, eh * 512:(eh + 1) * 512], op=add)
                    nc.sync.dma_start(
                        partials[ck][(stl % 2) * 128:(stl % 2) * 128 + 128,
                                     eh * 512:(eh + 1) * 512],
                        ob[:])
                if stl % 2 == 1:
                    nc.gpsimd.collective_compute(
                        "ReduceScatter", mybir.AluOpType.add,
                        replica_groups=RGROUPS,
                        ins=[partials[ck].opt()],
                        outs=[rs_outs[ck].opt()])
                    # bf16 RS output bounces through SBUF for the f32
                    # upconvert (DMA can't change dtype). The whole chain
                    # stays on the gpsimd queue, which the RS already
                    # serializes -- putting any link of it on the DVE/sync
                    # queues would stall those queues behind the RS.
                    rb16 = p2.tile([128, E], BF16, tag="rb16", bufs=1,
                                   name="rb16")
                    nc.gpsimd.dma_start(rb16[:], rs_outs[ck][:])
                    rb32 = p2.tile([128, E], F32, tag="rb32", bufs=1,
                                   name="rb32")
                    nc.gpsimd.tensor_copy(rb32[:], rb16[:])
                    nc.gpsimd.dma_start(
                        out.ap()[128 * ck:128 * (ck + 1), :], rb32[:])

            pending_out = [None]          # deferred output projection

            for qi, qb in enumerate(QBS):
                q0 = qb * 512
                nk = 4 * qb + 4           # k-tiles (causal)
                # prefetch x for the quarter whose projections interleave
                # into this attention block (gpsimd SWDGE queues: the sync
                # queues are busy with partial stores by then)
                if qb in (1, 2):
                    xts_q[qb + 1] = x_dma(qb + 1, eng=nc.gpsimd)
                    thunks = proj_thunks(qb + 1, xts_q[qb + 1])
                else:
                    thunks = []
                ti = [0]

                def fill(n):
                    for _ in range(n):
                        if ti[0] < len(thunks):
                            thunks[ti[0]]()
                            ti[0] += 1

                ctx4 = [p2.tile([128, 512], F32R, tag="ctxt", bufs=8,
                                name=f"ctxt{qb}_{i}") for i in range(4)]
                for h in range(HL):
                    t4, po = h >> 1, 64 * (h & 1)
                    ctx_ps = psp.tile([128, 512], F32, tag="ctx", bufs=2,
                                      name="ctxp")
                    attn_tiles = []
                    for p in range(nk // 2):   # k-tile pairs
                        sc_ps = psp.tile([128, 1024], F32, tag="sc",
                                         bufs=3, name="scp")
                        for u in range(2):
                            j = 2 * p + u
                            d = max(0, 128 * j - q0)
                            nc.tensor.matmul(
                                sc_ps[:, u * 512 + d:(u + 1) * 512],
                                kt_t[t4][:, j * 128:(j + 1) * 128],
                                qt8[h][:, q0 + d:q0 + 512],
                                start=True, stop=True)
                        at = p2.tile([128, 1024], BF16, tag="attn",
                                     bufs=6, name="attn")
                        d0 = max(0, 128 * 2 * p - q0)
                        with nc.allow_low_precision(reason="bf16"):
                            nc.scalar.activation(
                                at[:, d0:1024], sc_ps[:, d0:1024], Exp,
                                scale=SC)
                        for u in range(2):
                            j = 2 * p + u
                            d = 128 * j - q0
                            if d >= 0:    # diagonal: tri-mask the band
                                with nc.allow_low_precision(reason="bf16"):
                                    nc.vector.tensor_tensor(
                                        out=at[:, u * 512 + d:
                                               u * 512 + d + 128],
                                        in0=at[:, u * 512 + d:
                                               u * 512 + d + 128],
                                        in1=tri_sb[:], op=mult)
                        attn_tiles.append(at)
                    # PE fillers while exp of this head's tiles runs
                    fill(2)
                    if pending[0] is not None:
                        normalize()
                    if pending_out[0] is not None and 2 <= h <= 5:
                        outproj_piece(pending_out[0][0], pending_out[0][1],
                                      h - 2)
                        if h == 5:
                            pending_out[0] = None
                    for p in range(nk // 2):
                        at = attn_tiles[p]
                        for u in range(2):
                            j = 2 * p + u
                            d = max(0, 128 * j - q0)
                            nc.tensor.matmul(
                                ctx_ps[0:65, d:512],
                                v_t[j][:, h * 65:(h + 1) * 65],
                                at[:, u * 512 + d:(u + 1) * 512],
                                start=(j == 0), stop=(j == nk - 1))
                    pending[0] = (po, ctx4[t4], ctx_ps)
                fill(len(thunks))     # any leftovers
                normalize()
                pending_out[0] = (qb, ctx4)

            # final block's output projection (qb=0): 4 pieces + 2 RS
            for stl in range(4):
                outproj_piece(pending_out[0][0], pending_out[0][1], stl)

    nc.compile()
    return nc


def _in_maps(inputs):
    import ml_dtypes
    bf16 = ml_dtypes.bfloat16

    x = np.asarray(inputs["x"], dtype=np.float32)
    Wq = np.asarray(inputs["Wq"], dtype=np.float32)
    bq = np.asarray(inputs["bq"], dtype=np.float32)
    Wk = np.asarray(inputs["Wk"], dtype=np.float32)
    bk = np.asarray(inputs["bk"], dtype=np.float32)
    Wv = np.asarray(inputs["Wv"], dtype=np.float32)
    bv = np.asarray(inputs["bv"], dtype=np.float32)
    Wo = np.asarray(inputs["Wo"], dtype=np.float32)
    bo = np.asarray(inputs["bo"], dtype=np.float32)

    tri = np.triu(np.ones((128, 128), dtype=np.float32)).astype(bf16)
    e0 = np.zeros((128, 64), dtype=np.float32)
    e0[0, :] = 1.0
    maps = []
    for c in range(N_CORES):
        b, hb = c // 2, c % 2
        hs = slice(hb * HL, (hb + 1) * HL)
        wq_c = np.ascontiguousarray(
            Wq[hs].transpose(1, 0, 2).reshape(E, HL * Dh))
        wk_c = np.ascontiguousarray(
            Wk[hs].transpose(1, 0, 2).reshape(E, HL * Dh))
        wv_c = np.ascontiguousarray(
            Wv[hs].transpose(1, 0, 2).reshape(E, HL * Dh))
        bqk_c = np.concatenate(
            [bq[hs].reshape(4, 128).T, bk[hs].reshape(4, 128).T], axis=1)
        bvb_c = np.broadcast_to(
            bv[hs].reshape(1, HL * Dh), (128, HL * Dh)).copy()
        bob_c = (np.broadcast_to(bo, (128, E)).copy() if hb == 0
                 else np.zeros((128, E), dtype=np.float32))
        maps.append({
            "xT": np.ascontiguousarray(x[b].T),
            "wq": wq_c, "wk": wk_c, "wv": wv_c,
            "wo": np.ascontiguousarray(Wo[hb * 512:(hb + 1) * 512]),
            "bqk": np.ascontiguousarray(bqk_c),
            "bvb": bvb_c, "bob": bob_c,
            "maskt": tri,
            "e0sel": e0,
        })
    return maps


def kernel(**inputs) -> np.ndarray:
    from concourse.bass_utils import run_bass_kernel_spmd

    if "nc" not in _CACHE:
        _CACHE["nc"] = _build()
    nc = _CACHE["nc"]

    maps = _in_maps(inputs)
    res = run_bass_kernel_spmd(nc, maps, core_ids=list(range(N_CORES)),
                               **_CACHE.get("run_kwargs", {}))
    _CACHE["last_results"] = res

    # chunk ck holds summed rows [256*ck + 128*hb, +128) at
    # out[128*ck : 128*(ck+1)]
    out = np.empty((B, S, E), dtype=np.float32)
    for c in range(N_CORES):
        b, hb = c // 2, c % 2
        o = res.results[c]["out"]
        for ck in range(8):
            out[b, 256 * ck + 128 * hb:256 * ck + 128 * hb + 128, :] = \
                o[128 * ck:128 * (ck + 1)]
    return out


# revision 42
# speedup vs baseline: 1.1651x; 1.1651x over previous
"""Multi-head causal attention (B=4, S=2048, E=1024, H=16, Dh=64) on 8 TRN2
NeuronCores.

Sharding: core c -> batch b = c//2, head group hb = c%2 (8 heads each).
Each core computes Q/K/V projections for its 8 heads, causal softmax
attention, and a partial output projection over its 512 of the 1024
concat-head dims; a pairwise chunked ReduceScatter(add) sums the two head
groups, so core c returns 8 chunks of 128 sequence rows.

Schedule (single fused pipeline, PE kept continuously busy so the DVFS
p-state stays at max):
  P0, P1 (projection s-quarters) -> att(1) [P2 thunks interleaved] ->
  att(2) [P3 thunks + outproj(1) pieces] -> att(3) [outproj(2) pieces] ->
  att(0) [outproj(3) pieces] -> outproj(0) + final chunked RS (small tail).

Layouts on device:
  xT   [E=1024, S=2048] f32r (host-pretransposed x[b])
  QT   [128, S] bf16 per head: rows po..po+63 = Q_h^T, other 64 rows zero
       so every scores matmul contracts over K=128
  KT   [128 (2 heads x 64d), S] bf16 x 4 tiles
  V    [s-tile 128, 8 heads x 65] bf16 (65th col = ones -> softmax denom)
  scoresT [k-tile 128, q 512] f32 psum = KT_pair.T @ QT_pad; exp on ACT
  attn [128, 1024] bf16 (exp output; diag band masked by bf16 tri mult,
       4x DVE mode)
  ctxT [65, q 512] f32 psum accum over k-tiles = V_aug.T @ attnT
       (row 64 = denom)
  out  [s-tile 128, e 512] accum over 4 f-tiles = ctxT.T @ Wo (f32r)

Softmax runs unnormalized; normalization is batched per head-pair: two
reciprocals read the denom rows straight from PSUM, one PE matmul
broadcasts both recips to 128 partitions via a [2,128] selector, then two
[64,512] DVE mults write the normalized ctx tiles (f32r).
"""

import numpy as np

B, S, E = 4, 2048, 1024
H, Dh = 16, 64
HL = 8          # heads per core
N_CORES = 8
SC = 0.125      # 1/sqrt(Dh)

_CACHE = {}


def _build():
    import concourse.bacc as bacc
    import concourse.mybir as mybir
    import concourse.tile as tile

    F32 = mybir.dt.float32
    F32R = mybir.dt.float32r
    BF16 = mybir.dt.bfloat16
    Exp = mybir.ActivationFunctionType.Exp
    mult = mybir.AluOpType.mult
    add = mybir.AluOpType.add

    nc = bacc.Bacc("TRN2", target_bir_lowering=False, debug=False)

    xT = nc.dram_tensor("xT", [E, S], F32R, kind="ExternalInput")
    wq = nc.dram_tensor("wq", [E, 512], F32R, kind="ExternalInput")
    wk = nc.dram_tensor("wk", [E, 512], F32R, kind="ExternalInput")
    wv = nc.dram_tensor("wv", [E, 512], F32R, kind="ExternalInput")
    wo = nc.dram_tensor("wo", [512, E], F32R, kind="ExternalInput")
    bqk = nc.dram_tensor("bqk", [128, 8], F32, kind="ExternalInput")
    bvb = nc.dram_tensor("bvb", [128, 512], F32, kind="ExternalInput")
    bob = nc.dram_tensor("bob", [128, E], F32, kind="ExternalInput")
    maskt = nc.dram_tensor("maskt", [128, 128], BF16, kind="ExternalInput")
    e0sel = nc.dram_tensor("e0sel", [128, 64], F32R, kind="ExternalInput")
    out = nc.dram_tensor("out", [S // 2, E], F32, kind="ExternalOutput")

    NST = S // 128          # 16 s-tiles of 128
    NET = E // 128          # 8 e-tiles
    QBS = [1, 2, 3, 0]      # q-block processing order (small block last)

    with tile.TileContext(nc) as tc:
        with (
            tc.tile_pool(name="persist", bufs=1) as pp,
            tc.tile_pool(name="psum", bufs=1, space="PSUM") as psp,
            tc.tile_pool(name="dram", bufs=1, space="DRAM") as dram,
            tc.tile_pool(name="ph1", bufs=1) as p1,
            tc.tile_pool(name="ph2", bufs=1) as p2,
        ):
            # ---- persistent tiles ----
            qt8 = [pp.tile([128, S], BF16, tag="qt", bufs=8, name=f"qt{i}")
                   for i in range(HL)]
            kt_t = [pp.tile([128, S], BF16, tag="kt", bufs=4, name=f"kt{i}")
                    for i in range(4)]
            v_t = [pp.tile([128, HL * 65], BF16, tag="v", bufs=NST,
                           name=f"v{i}") for i in range(NST)]

            bqk_sb = pp.tile([128, 8], F32, tag="bqk")
            bvb_sb = pp.tile([128, 512], F32, tag="bvb")
            tri_sb = pp.tile([128, 128], BF16, tag="mask", name="tri")
            e0_sb = pp.tile([128, 64], F32R, tag="e0", name="e0")
            rfull = [pp.tile([128, 512], F32R, tag=f"rfull{i}",
                             name=f"rfull{i}") for i in range(2)]
            bob_sb = pp.tile([128, E], F32, tag="bob", name="bob")
            wos = [pp.tile([128, E], F32R, tag="wo", bufs=4, name=f"wo{ft}")
                   for ft in range(4)]

            partials = [dram.tile([256, E], BF16, tag="partial", bufs=8,
                                  name=f"partial{i}") for i in range(8)]
            rs_outs = [dram.tile([128, E], BF16, tag="rsout", bufs=8,
                                 name=f"rsout{i}") for i in range(8)]
            RGROUPS = [[0, 1], [2, 3], [4, 5], [6, 7]]

            # ---- DMA helpers ----
            def x_dma(qt, eng=None):
                """Issue the 8 xT tile DMAs for s-quarter qt."""
                if eng is None:
                    eng = nc.sync
                s0 = qt * 512
                tiles = []
                for e in range(NET):
                    t = p1.tile([128, 512], F32R, tag="xt", bufs=12,
                                name=f"xt{qt}_{e}")
                    eng.dma_start(
                        t[:], xT.ap()[e * 128:(e + 1) * 128, s0:s0 + 512])
                    tiles.append(t)
                return tiles

            # first quarter's x, interleaved with wq (consumed first)
            wt = {0: [], 1: [], 2: []}
            xts_q = {}
            s0 = 0
            xts_q[0] = []
            for e in range(NET):
                t = p1.tile([128, 512], F32R, tag="xt", bufs=12,
                            name=f"xt0_{e}")
                w = p1.tile([128, 512], F32R, tag="w", bufs=24,
                            name=f"w0_{e}")
                nc.sync.dma_start(t[:], xT.ap()[e * 128:(e + 1) * 128, 0:512])
                nc.sync.dma_start(w[:], wq.ap()[e * 128:(e + 1) * 128, :])
                xts_q[0].append(t)
                wt[0].append(w)
            nc.sync.dma_start(bqk_sb[:], bqk.ap())
            nc.sync.dma_start(bvb_sb[:], bvb.ap())
            # wk/wv ride the scalar/vector engine DMA queues so the sync
            # queues stay dedicated to the x stream (startup is DMA-bound)
            for wi, wd in [(1, wk), (2, wv)]:
                for e in range(NET):
                    w = p1.tile([128, 512], F32R, tag="w", bufs=24,
                                name=f"w{wi}_{e}")
                    eng = nc.scalar if e < 4 else nc.gpsimd
                    eng.dma_start(w[:], wd.ap()[e * 128:(e + 1) * 128, :])
                    wt[wi].append(w)
            xts_q[1] = x_dma(1)
            nc.scalar.dma_start(tri_sb[:], maskt.ap())
            nc.scalar.dma_start(e0_sb[:], e0sel.ap())
            nc.scalar.dma_start(bob_sb[:], bob.ap())
            for ft in range(4):
                nc.scalar.dma_start(wos[ft][:],
                                    wo.ap()[ft * 128:(ft + 1) * 128, :])

            # ---- projection emitters ----
            def proj_q_tile(qt, t4, xts):
                cols = slice(qt * 512, qt * 512 + 512)
                ws = wt[0]
                ps = psp.tile([128, 1024], F32, tag="sc", bufs=3,
                              name="accp")
                for e in range(NET):
                    nc.tensor.matmul(
                        ps[:, 0:512],
                        ws[e][:, t4 * 128:(t4 + 1) * 128],
                        xts[e][:],
                        start=(e == 0), stop=(e == NET - 1))
                with nc.allow_low_precision(reason="bf16"):
                    nc.vector.tensor_scalar_add(
                        qt8[2 * t4][0:64, cols], ps[0:64, 0:512],
                        bqk_sb[0:64, t4:t4 + 1])
                    nc.vector.tensor_scalar_add(
                        qt8[2 * t4 + 1][64:128, cols], ps[64:128, 0:512],
                        bqk_sb[64:128, t4:t4 + 1])

            def proj_k_tile(qt, t4, xts):
                cols = slice(qt * 512, qt * 512 + 512)
                ws = wt[1]
                ps = psp.tile([128, 1024], F32, tag="sc", bufs=3,
                              name="accp")
                for e in range(NET):
                    nc.tensor.matmul(
                        ps[:, 0:512],
                        ws[e][:, t4 * 128:(t4 + 1) * 128],
                        xts[e][:],
                        start=(e == 0), stop=(e == NET - 1))
                with nc.allow_low_precision(reason="bf16"):
                    nc.vector.tensor_scalar_add(
                        kt_t[t4][:, cols], ps[:, 0:512],
                        bqk_sb[:, 4 + t4:5 + t4])

            def proj_v_tile(qt, st, xts):
                gst = qt * 4 + st
                ws = wt[2]
                ps = psp.tile([128, 1024], F32, tag="sc", bufs=3,
                              name="accp")
                for e in range(NET):
                    nc.tensor.matmul(
                        ps[:, 0:512],
                        xts[e][:, st * 128:(st + 1) * 128],
                        ws[e][:],
                        start=(e == 0), stop=(e == NET - 1))
                vt = v_t[gst]
                # contiguous whole-tile memset supplies the per-head ones
                # columns (col 64); the V write below overwrites cols 0-63
                nc.vector.memset(vt[:], 1.0)
                v3 = vt[:].rearrange("p (h d) -> p h d", h=HL, d=65)
                with nc.allow_low_precision(reason="bf16"):
                    nc.vector.tensor_tensor(
                        out=v3[:, :, 0:64],
                        in0=ps[:, 0:512].rearrange(
                            "p (h d) -> p h d", h=HL, d=64),
                        in1=bvb_sb[:].rearrange(
                            "p (h d) -> p h d", h=HL, d=64),
                        op=add)

            def proj_thunks(qt, xts):
                th = []
                for t4 in range(4):
                    th.append(lambda qt=qt, t4=t4: proj_q_tile(qt, t4, xts))
                for t4 in range(4):
                    th.append(lambda qt=qt, t4=t4: proj_k_tile(qt, t4, xts))
                for st in range(4):
                    th.append(lambda qt=qt, st=st: proj_v_tile(qt, st, xts))
                return th

            # ---- eager quarters 0 and 1 ----
            for thunk in proj_thunks(0, xts_q[0]):
                thunk()
            # zero the pad rows of the per-head QT tiles (emitted after P0
            # so the DVE runs them behind P0's bias-adds, not before)
            for h in range(HL):
                pad = 64 - 64 * (h & 1)   # the *other* 64 rows
                nc.vector.memset(qt8[h][pad:pad + 64, :], 0.0)
            # zero rfull rows once (garbage x 0 via a clean source; memset
            # can't write float32r)
            for t in rfull:
                with nc.allow_low_precision(reason="f32r"):
                    nc.vector.tensor_scalar_mul(t[:], xts_q[0][0][:], 0.0)
            for thunk in proj_thunks(1, xts_q[1]):
                thunk()

            # ---- attention + outproj pipeline ----
            pending = [None]              # deferred normalization
            nblk = [0]

            def normalize():
                po, ctxd, ctx_ps = pending[0]
                rf = rfull[nblk[0] % 2]
                nblk[0] += 1
                dn = p2.tile([1, 512], F32, tag="dn", bufs=1, name="dn")
                nc.vector.tensor_copy(dn[:], ctx_ps[64:65, :])
                rc = p2.tile([1, 512], F32, tag="rc", bufs=1, name="rc")
                nc.vector.reciprocal_approx_fast(out=rc[:], in_=dn[:])
                with nc.allow_low_precision(reason="f32r"):
                    nc.vector.tensor_copy(rf[0:1, :], rc[:])
                rb_ps = psp.tile([128, 1024], F32, tag="sc", bufs=3,
                                 name="rbp")
                nc.tensor.matmul(rb_ps[0:64, 0:512], e0_sb[:], rf[:],
                                 start=True, stop=True)
                recip_b = p2.tile([64, 512], F32, tag="recipb", bufs=1,
                                  name="recipb")
                nc.vector.tensor_copy(recip_b[:], rb_ps[0:64, 0:512])
                with nc.allow_low_precision(reason="f32r"):
                    nc.vector.tensor_tensor(
                        out=ctxd[po:po + 64, :],
                        in0=ctx_ps[0:64, :], in1=recip_b[:], op=mult)
                pending[0] = None

            def outproj_piece(qb, ctx4, stl):
                ck = 2 * qb + stl // 2
                for eh in range(2):
                    ps = psp.tile([128, 1024], F32, tag="sc", bufs=3,
                                  name="accp3")
                    for ft in range(4):
                        nc.tensor.matmul(
                            ps[:, 0:512],
                            ctx4[ft][:, stl * 128:(stl + 1) * 128],
                            wos[ft][:, eh * 512:(eh + 1) * 512],
                            start=(ft == 0), stop=(ft == 3))
                    ob = p2.tile([128, 512], BF16, tag="outp", bufs=3,
                                 name="outp")
                    with nc.allow_low_precision(reason="bf16"):
                        nc.vector.tensor_tensor(
                            out=ob[:], in0=ps[:, 0:512],
                            in1=bob_sb[:, eh * 512:(eh + 1) * 512], op=add)
                    nc.sync.dma_start(
                        partials[ck][(stl % 2) * 128:(stl % 2) * 128 + 128,
                                     eh * 512:(eh + 1) * 512],
                        ob[:])
                if stl % 2 == 1:
                    nc.gpsimd.collective_compute(
                        "ReduceScatter", mybir.AluOpType.add,
                        replica_groups=RGROUPS,
                        ins=[partials[ck].opt()],
                        outs=[rs_outs[ck].opt()])
                    # bf16 RS output bounces through SBUF for the f32
                    # upconvert (DMA can't change dtype). The dma-in rides
                    # the gpsimd queue (already blocked by the RS); the DVE
                    # cast is DEFERRED so the DVE queue never waits on a
                    # still-running RS.
                    rb16 = p2.tile([128, E], BF16, tag="rb16", bufs=2,
                                   name="rb16")
                    nc.gpsimd.dma_start(rb16[:], rs_outs[ck][:])
                    pending_bounce.append((ck, rb16))

            pending_out = [None]          # deferred output projection
            pending_bounce = []           # deferred RS-output upconverts

            def drain_bounce(n):
                for _ in range(n):
                    if not pending_bounce:
                        return
                    ck, rb16 = pending_bounce.pop(0)
                    rb32 = p2.tile([128, E], F32, tag="rb32", bufs=1,
                                   name="rb32")
                    nc.vector.tensor_copy(rb32[:], rb16[:])
                    nc.sync.dma_start(
                        out.ap()[128 * ck:128 * (ck + 1), :], rb32[:])

            for qi, qb in enumerate(QBS):
                q0 = qb * 512
                nk = 4 * qb + 4           # k-tiles (causal)
                # prefetch x for the quarter whose projections interleave
                # into this attention block (gpsimd SWDGE queues: the sync
                # queues are busy with partial stores by then)
                if qb in (1, 2):
                    xts_q[qb + 1] = x_dma(qb + 1, eng=nc.gpsimd)
                    thunks = proj_thunks(qb + 1, xts_q[qb + 1])
                else:
                    thunks = []
                ti = [0]

                def fill(n):
                    for _ in range(n):
                        if ti[0] < len(thunks):
                            thunks[ti[0]]()
                            ti[0] += 1

                ctx4 = [p2.tile([128, 512], F32R, tag="ctxt", bufs=8,
                                name=f"ctxt{qb}_{i}") for i in range(4)]
                for h in range(HL):
                    t4, po = h >> 1, 64 * (h & 1)
                    ctx_ps = psp.tile([128, 512], F32, tag="ctx", bufs=2,
                                      name="ctxp")
                    attn_tiles = []
                    for p in range(nk // 2):   # k-tile pairs
                        sc_ps = psp.tile([128, 1024], F32, tag="sc",
                                         bufs=3, name="scp")
                        for u in range(2):
                            j = 2 * p + u
                            d = max(0, 128 * j - q0)
                            nc.tensor.matmul(
                                sc_ps[:, u * 512 + d:(u + 1) * 512],
                                kt_t[t4][:, j * 128:(j + 1) * 128],
                                qt8[h][:, q0 + d:q0 + 512],
                                start=True, stop=True)
                        at = p2.tile([128, 1024], BF16, tag="attn",
                                     bufs=5, name="attn")
                        d0 = max(0, 128 * 2 * p - q0)
                        with nc.allow_low_precision(reason="bf16"):
                            nc.scalar.activation(
                                at[:, d0:1024], sc_ps[:, d0:1024], Exp,
                                scale=SC)
                        for u in range(2):
                            j = 2 * p + u
                            d = 128 * j - q0
                            if d >= 0:    # diagonal: tri-mask the band
                                with nc.allow_low_precision(reason="bf16"):
                                    nc.vector.tensor_tensor(
                                        out=at[:, u * 512 + d:
                                               u * 512 + d + 128],
                                        in0=at[:, u * 512 + d:
                                               u * 512 + d + 128],
                                        in1=tri_sb[:], op=mult)
                        attn_tiles.append(at)
                    # PE fillers while exp of this head's tiles runs
                    fill(2)
                    if pending[0] is not None:
                        normalize()
                    if pending_out[0] is not None and 2 <= h <= 5:
                        outproj_piece(pending_out[0][0], pending_out[0][1],
                                      h - 2)
                        if h == 5:
                            pending_out[0] = None
                    if h in (1, 7):
                        drain_bounce(1)
                    for p in range(nk // 2):
                        at = attn_tiles[p]
                        for u in range(2):
                            j = 2 * p + u
                            d = max(0, 128 * j - q0)
                            nc.tensor.matmul(
                                ctx_ps[0:65, d:512],
                                v_t[j][:, h * 65:(h + 1) * 65],
                                at[:, u * 512 + d:(u + 1) * 512],
                                start=(j == 0), stop=(j == nk - 1))
                    pending[0] = (po, ctx4[t4], ctx_ps)
                fill(len(thunks))     # any leftovers
                normalize()
                pending_out[0] = (qb, ctx4)

            # final block's output projection (qb=0): 4 pieces + 2 RS
            for stl in range(4):
                outproj_piece(pending_out[0][0], pending_out[0][1], stl)
            drain_bounce(8)

    nc.compile()
    return nc


def _in_maps(inputs):
    import ml_dtypes
    bf16 = ml_dtypes.bfloat16

    x = np.asarray(inputs["x"], dtype=np.float32)
    Wq = np.asarray(inputs["Wq"], dtype=np.float32)
    bq = np.asarray(inputs["bq"], dtype=np.float32)
    Wk = np.asarray(inputs["Wk"], dtype=np.float32)
    bk = np.asarray(inputs["bk"], dtype=np.float32)
    Wv = np.asarray(inputs["Wv"], dtype=np.float32)
    bv = np.asarray(inputs["bv"], dtype=np.float32)
    Wo = np.asarray(inputs["Wo"], dtype=np.float32)
    bo = np.asarray(inputs["bo"], dtype=np.float32)

    tri = np.triu(np.ones((128, 128), dtype=np.float32)).astype(bf16)
    e0 = np.zeros((128, 64), dtype=np.float32)
    e0[0, :] = 1.0
    maps = []
    for c in range(N_CORES):
        b, hb = c // 2, c % 2
        hs = slice(hb * HL, (hb + 1) * HL)
        wq_c = np.ascontiguousarray(
            Wq[hs].transpose(1, 0, 2).reshape(E, HL * Dh))
        wk_c = np.ascontiguousarray(
            Wk[hs].transpose(1, 0, 2).reshape(E, HL * Dh))
        wv_c = np.ascontiguousarray(
            Wv[hs].transpose(1, 0, 2).reshape(E, HL * Dh))
        bqk_c = np.concatenate(
            [bq[hs].reshape(4, 128).T, bk[hs].reshape(4, 128).T], axis=1)
        bvb_c = np.broadcast_to(
            bv[hs].reshape(1, HL * Dh), (128, HL * Dh)).copy()
        bob_c = (np.broadcast_to(bo, (128, E)).copy() if hb == 0
                 else np.zeros((128, E), dtype=np.float32))
        maps.append({
            "xT": np.ascontiguousarray(x[b].T),
            "wq": wq_c, "wk": wk_c, "wv": wv_c,
            "wo": np.ascontiguousarray(Wo[hb * 512:(hb + 1) * 512]),
            "bqk": np.ascontiguousarray(bqk_c),
            "bvb": bvb_c, "bob": bob_c,
            "maskt": tri,
            "e0sel": e0,
        })
    return maps


def kernel(**inputs) -> np.ndarray:
    from concourse.bass_utils import run_bass_kernel_spmd

    if "nc" not in _CACHE:
        _CACHE["nc"] = _build()
    nc = _CACHE["nc"]

    maps = _in_maps(inputs)
    res = run_bass_kernel_spmd(nc, maps, core_ids=list(range(N_CORES)),
                               **_CACHE.get("run_kwargs", {}))
    _CACHE["last_results"] = res

    # chunk ck holds summed rows [256*ck + 128*hb, +128) at
    # out[128*ck : 128*(ck+1)]
    out = np.empty((B, S, E), dtype=np.float32)
    for c in range(N_CORES):
        b, hb = c // 2, c % 2
        o = res.results[c]["out"]
        for ck in range(8):
            out[b, 256 * ck + 128 * hb:256 * ck + 128 * hb + 128, :] = \
                o[128 * ck:128 * (ck + 1)]
    return out
